# revision 2
# baseline (speedup 1.0000x reference)
"""Trainium2 Bass kernel for CustomLossWithCovariance.

loss = abs(logdet(sigma) + mean_b[(p_b - t_b)^T sigma^{-1} (p_b - t_b)])

Only the 3x3 Gram matrix G = sum_b d_b d_b^T (d = pred - targ) requires
touching the [B, 3] data; the device computes per-core partial pair-sums
of G, and the host finishes with the tiny 3x3 algebra:
    mean_mahalanobis = <sigma_inv, G> / B
    loss = |logdet(sigma) + mean_mahalanobis|

Sharding: data-parallel over the batch across 8 NeuronCores (each core
streams a contiguous [B/8, 3] shard; partial sums gathered on host).

Per-core device kernel (raw Bacc, manual semaphores — see
build_gram_kernel_raw; build_gram_kernel is the Tile-framework
baseline kept for reference). Per tile of [128, 2M]:
  - DMA both halves (pred | targ) flat-contiguous in one dma_start
  - DVE: d = pred - targ, in-place into the pred half (unit-stride fp32)
  - ACT: Square(d_i) with accum_out -> per-partition sums of d_i^2
  - DVE: scalar_tensor_tensor(d_i * d_j, accum_out) -> cross sums
  (component APs are stride-3 views of the flat tiles, grouped 4 tiles
  per reduce instruction to amortize fixed costs)
"""

import numpy as np

import concourse.bass as bass
import concourse.bacc as bacc
import concourse.mybir as mybir
from concourse import tile
from concourse.bass_utils import run_bass_kernel_spmd

N_CORES = 8
B_FULL = 8388608
P = 128

_PAIRS = [(0, 1), (0, 2), (1, 2)]


def build_gram_kernel(n_rows: int, n_tiles: int, use_act: bool = True):
    """Build the per-core Bass module.

    Input: pt [2, n_rows, 3] f32 (pred stacked with targ)
    Output: partials [128, 6 * n_tiles] f32
        col t*3+i            : sum over this tile/partition of d_i^2
        col 3*n_tiles + t*3+k: sum of d_i*d_j for pair k in _PAIRS
    """
    assert n_rows % (P * n_tiles) == 0
    r = n_rows // (P * n_tiles)  # rows per partition per tile
    m = 3 * r                    # flat f32 elements per partition per tile
    f32 = mybir.dt.float32

    # Bacc (not plain Bass): its compile() pass legalizes semaphore waits
    # (each TRN2 instruction holds at most one wait slot).
    nc = bacc.Bacc("TRN2", target_bir_lowering=False, debug=False)
    pt = nc.dram_tensor("pt", [2, n_rows, 3], f32, kind="ExternalInput")
    out = nc.dram_tensor("partials", [P, 6 * n_tiles], f32, kind="ExternalOutput")

    # [t][p][w(2), m] — per tile/partition: pred chunk and targ chunk, each
    # m contiguous f32 in DRAM.
    pt_v = pt[:].rearrange("w (t p r) c -> t p w (r c)", t=n_tiles, p=P)

    with tile.TileContext(nc) as tc:
        with (
            tc.tile_pool(name="io", bufs=3) as io_pool,
            tc.tile_pool(name="dve_scr", bufs=2) as dve_scr,
            tc.tile_pool(name="act_scr", bufs=2) as act_scr,
            tc.tile_pool(name="acc", bufs=1) as acc_pool,
        ):
            acc_sq = acc_pool.tile([P, 3 * n_tiles], f32)
            acc_cr = acc_pool.tile([P, 3 * n_tiles], f32)

            for t in range(n_tiles):
                buf = io_pool.tile([P, 2 * m], f32, tag="buf")
                nc.sync.dma_start(
                    out=buf[:].rearrange("p (w m) -> p w m", w=2),
                    in_=pt_v[t],
                )

                # In-place: d = pred - targ, overwriting the pred half.
                nc.vector.tensor_tensor(
                    out=buf[:, 0:m],
                    in0=buf[:, 0:m],
                    in1=buf[:, m : 2 * m],
                    op=mybir.AluOpType.subtract,
                )
                d3 = buf[:, 0:m].rearrange("p (r c) -> p c r", c=3)

                # Diagonal sums on the scalar engine (Square + accum_out),
                # overlapping with the DVE cross-products.
                if use_act:
                    for i in range(3):
                        sq = act_scr.tile([P, r], f32, tag="sq")
                        nc.scalar.activation(
                            out=sq[:],
                            in_=d3[:, i, :],
                            func=mybir.ActivationFunctionType.Square,
                            accum_out=acc_sq[:, t * 3 + i : t * 3 + i + 1],
                        )
                else:
                    for i in range(3):
                        sq = dve_scr.tile([P, r], f32, tag="pr")
                        nc.vector.scalar_tensor_tensor(
                            out=sq[:],
                            in0=d3[:, i, :],
                            scalar=1.0,
                            in1=d3[:, i, :],
                            op0=mybir.AluOpType.mult,
                            op1=mybir.AluOpType.mult,
                            accum_out=acc_sq[:, t * 3 + i : t * 3 + i + 1],
                        )
                # Cross sums: fused multiply+reduce on DVE
                # (scalar_tensor_tensor: out = (in0 * 1.0) * in1, accum = sum).
                for k, (i, j) in enumerate(_PAIRS):
                    pr = dve_scr.tile([P, r], f32, tag="pr")
                    nc.vector.scalar_tensor_tensor(
                        out=pr[:],
                        in0=d3[:, i, :],
                        scalar=1.0,
                        in1=d3[:, j, :],
                        op0=mybir.AluOpType.mult,
                        op1=mybir.AluOpType.mult,
                        accum_out=acc_cr[:, t * 3 + k : t * 3 + k + 1],
                    )

            nc.sync.dma_start(out=out[:, 0 : 3 * n_tiles], in_=acc_sq[:])
            nc.sync.dma_start(out=out[:, 3 * n_tiles : 6 * n_tiles], in_=acc_cr[:])

    nc.compile()
    return nc


def build_gram_kernel_raw(n_rows: int, n_tiles: int = 32, n_bufs: int = 24,
                          group: int = 4, skip_exit_barrier: bool = True):
    """Raw-Bacc variant: manual semaphores, no TileContext.

    Skips Tile's prologue/epilogue (drain + two all-engine EVSEM
    barriers, ~16 us) — the only sync needed is a three-semaphore chain:
    DMA loads (one HWDGE ring) -> DVE -> ACT.

    The ring of tile buffers lives in ONE SBUF tensor so the fused
    multiply-reduces can span `group` consecutive tiles with a single
    instruction (free-dim AP [group, r]) — amortizing the per-op fixed
    cost and the accumulator-drain, which keeps both compute engines
    well under the DMA pace.

    Input: pt [2, n_rows, 3] f32. Output: partials [128, 6 * n_groups]
    (same slot layout as build_gram_kernel, with n_groups slots).
    """
    assert n_tiles % group == 0 and n_bufs % group == 0
    assert n_rows % (P * n_tiles) == 0
    n_groups = n_tiles // group
    r = n_rows // (P * n_tiles)
    m = 3 * r
    f32 = mybir.dt.float32

    nc = bacc.Bacc("TRN2", target_bir_lowering=False, debug=False)
    pt = nc.dram_tensor("pt", [2, n_rows, 3], f32, kind="ExternalInput")
    out = nc.dram_tensor("partials", [P, 6 * n_groups], f32, kind="ExternalOutput")
    pt_v = pt[:].rearrange("w (t p r) c -> t p w (r c)", t=n_tiles, p=P)

    ring = nc.alloc_sbuf_tensor("ring", [P, n_bufs * 2 * m], f32).ap()

    def buf(t):
        s = t % n_bufs
        return ring[:, s * 2 * m : (s + 1) * 2 * m]

    def dgroup(g, i):
        # component i of the diff halves of tiles 4g..4g+3: [128, group, r]
        s0 = (g * group) % n_bufs
        w = ring[:, s0 * 2 * m : (s0 + group) * 2 * m]
        return w.rearrange("p (t w r c) -> p t w c r", t=group, w=2, c=3)[:, :, 0, i, :]

    acc_sq = nc.alloc_sbuf_tensor("acc_sq", [P, 3 * n_groups], f32).ap()
    acc_cr = nc.alloc_sbuf_tensor("acc_cr", [P, 3 * n_groups], f32).ap()
    # Rotated scratch (dead stores of the fused ops), 2 groups deep so each
    # group's single stale semaphore wait also covers the scratch WAW from
    # two groups back.
    pr_scrs = [
        nc.alloc_sbuf_tensor(f"pr_scr{k}", [P, group * r], f32).ap() for k in range(6)
    ]
    sq_scrs = [
        nc.alloc_sbuf_tensor(f"sq_scr{k}", [P, group * r], f32).ap() for k in range(6)
    ]

    # One DMA-completion semaphore per ring buffer: a single shared sem
    # would be unsound — each dma_start is split across 16 SDMA engines
    # whose sub-completions interleave across in-flight DMAs.
    dma_sems = [nc.alloc_semaphore(f"dma_sem{i}") for i in range(n_bufs)]
    out_sem = nc.alloc_semaphore("out_sem")
    dve_sem = nc.alloc_semaphore("dve_sem")
    act_sem = nc.alloc_semaphore("act_sem")

    # DVE emission order: subs run ahead; the grouped multiply-reduces for
    # group g are emitted after sub(4g+4) so their drain-wait on the last
    # sub of the group is already satisfied when it executes (DVE writes
    # drain asynchronously). Only the last group trails the final sub.
    dve_order = []
    for t in range(n_tiles):
        dve_order.append(("sub", t))
        if t % group == 0 and t >= group:
            # one sub of stagger after the group's last sub
            dve_order.append(("stt", t // group - 1))
    dve_order.append(("stt", n_groups - 1))
    sub_done, sttg_done = {}, {}
    v = 0
    for kind, x in dve_order:
        if kind == "sub":
            v += 1
            sub_done[x] = v
        else:
            v += 3
            sttg_done[x] = v

    # Output chunks: flush finished accumulator columns while later tiles
    # still stream, so the tail only waits on the last small chunk.
    chunk = max(1, n_groups // 2)
    chunks = [(c, min(c + chunk, n_groups)) for c in range(0, n_groups, chunk)]

    import contextlib

    @contextlib.contextmanager
    def _block():
        # no_gpsimd_drain=True emits per-engine drains explicitly and then a
        # sem-only all-engine butterfly. The butterfly only delays NEFF end
        # (outputs are already fenced by the sequencer's out_sem wait), so
        # optionally no-op it during Block.__exit__.
        with nc.Block(no_gpsimd_drain=True) as blk:
            try:
                yield blk
            finally:
                if skip_exit_barrier:
                    nc.all_engine_barrier = lambda **kw: None
        if skip_exit_barrier:
            del nc.all_engine_barrier  # restore class method

    with _block() as block:

        @block.sync
        def _(sync):
            for t in range(n_tiles):
                if t >= n_bufs:
                    # ring reuse: all consumers of the buffer's previous
                    # occupant (tile t - n_bufs) must be done
                    prev = t - n_bufs
                    sync.wait_ge(dve_sem, sttg_done[prev // group])
                    sync.wait_ge(act_sem, 3 * (prev // group + 1))
                sync.dma_start(
                    out=buf(t).rearrange("p (w m) -> p w m", w=2),
                    in_=pt_v[t],
                ).then_inc(dma_sems[t % n_bufs], 16)
            n_out = 0
            for lo, hi in chunks:
                sync.wait_ge(act_sem, 3 * hi)
                sync.dma_start(
                    out=out[:, 3 * lo : 3 * hi], in_=acc_sq[:, 3 * lo : 3 * hi]
                ).then_inc(out_sem, 16)
                sync.wait_ge(dve_sem, sttg_done[hi - 1])
                sync.dma_start(
                    out=out[:, 3 * (n_groups + lo) : 3 * (n_groups + hi)],
                    in_=acc_cr[:, 3 * lo : 3 * hi],
                ).then_inc(out_sem, 16)
                n_out += 32
            sync.wait_ge(out_sem, n_out)

        @block.vector
        def _(vector):
            for kind, x in dve_order:
                if kind == "sub":
                    b = buf(x)
                    vector.wait_ge(dma_sems[x % n_bufs], 16 * (x // n_bufs + 1))
                    vector.tensor_tensor(
                        out=b[:, 0:m],
                        in0=b[:, 0:m],
                        in1=b[:, m : 2 * m],
                        op=mybir.AluOpType.subtract,
                    ).then_inc(dve_sem, 1)
                else:
                    vector.wait_ge(dve_sem, sub_done[(x + 1) * group - 1])
                    for k, (i, j) in enumerate(_PAIRS):
                        vector.scalar_tensor_tensor(
                            out=pr_scrs[(x % 2) * 3 + k][:].rearrange(
                                "p (t r) -> p t r", t=group
                            ),
                            in0=dgroup(x, i),
                            scalar=1.0,
                            in1=dgroup(x, j),
                            op0=mybir.AluOpType.mult,
                            op1=mybir.AluOpType.mult,
                            accum_out=acc_cr[:, x * 3 + k : x * 3 + k + 1],
                        ).then_inc(dve_sem, 1)

        @block.scalar
        def _(scalar):
            for g in range(n_groups):
                scalar.wait_ge(dve_sem, sub_done[(g + 1) * group - 1])
                if g >= 2:
                    # scratch slot reuse from two groups back
                    scalar.wait_ge(act_sem, 3 * (g - 1))
                for i in range(3):
                    scalar.activation(
                        out=sq_scrs[(g % 2) * 3 + i][:].rearrange(
                            "p (t r) -> p t r", t=group
                        ),
                        in_=dgroup(g, i),
                        func=mybir.ActivationFunctionType.Square,
                        accum_out=acc_sq[:, g * 3 + i : g * 3 + i + 1],
                    ).then_inc(act_sem, 1)

    nc.compile()
    return nc

def build_gram_kernel_v2(n_rows: int, n_tiles: int = 32, n_bufs: int = 24,
                         group: int = 4, n_dbufs: int = 12,
                         n_scalar_dmas: int = 8, act_squares: int = 3,
                         skip_exit_barrier: bool = True):
    """bf16-deinterleaved variant of build_gram_kernel_raw.

    The fp32 per-tile subtract writes d = pred - targ as bf16 with the
    three vector components DEINTERLEAVED (each component a unit-stride
    block) into a small d-ring.  The multiply-reduces then read bf16 at
    step 1, which unlocks the DVE 2x packed perf mode (fp32/stride-3 in
    the baseline capped DVE at ~85-103 elem/ns and made compute lag the
    402 GB/s DMA stream by ~8 us).

    Other deltas vs the baseline:
      - input-ring reuse only waits on the SUB of the evicted tile (the
        crosses read the d-ring, not the input ring), so the DMA queue
        never stalls on the reduce tail;
      - the first `n_scalar_dmas` tile loads are issued from the scalar
        engine's HWDGE ring in parallel with the sync engine's, halving
        the issue-rate-limited ramp;
      - the accumulator is laid out group-major ([sq0..2 cr0..2] per
        group) so each output flush is ONE dma, and the final flush
        covers only the last group's 6 columns.

    Output: partials [128, 6 * n_groups], col 6g+i = sum d_i^2 of group
    g for i<3, col 6g+3+k = sum d_i*d_j for pair k.
    """
    assert n_tiles % group == 0 and n_bufs % group == 0
    assert n_dbufs % group == 0 and n_dbufs >= 2 * group
    assert n_rows % (P * n_tiles) == 0
    assert 0 <= act_squares <= 3
    n_groups = n_tiles // group
    r = n_rows // (P * n_tiles)
    m = 3 * r
    f32, bf16 = mybir.dt.float32, mybir.dt.bfloat16

    nc = bacc.Bacc("TRN2", target_bir_lowering=False, debug=False)
    pt = nc.dram_tensor("pt", [2, n_rows, 3], f32, kind="ExternalInput")
    out = nc.dram_tensor("partials", [P, 6 * n_groups], f32, kind="ExternalOutput")
    pt_v = pt[:].rearrange("w (t p r) c -> t p w (r c)", t=n_tiles, p=P)

    ring = nc.alloc_sbuf_tensor("ring", [P, n_bufs * 2 * m], f32).ap()
    dring = nc.alloc_sbuf_tensor("dring", [P, n_dbufs * m], bf16).ap()
    acc = nc.alloc_sbuf_tensor("acc", [P, 6 * n_groups], f32).ap()

    def buf(t):
        s = t % n_bufs
        return ring[:, s * 2 * m : (s + 1) * 2 * m]

    def sub_views(t):
        b = buf(t)
        p_v = b[:, 0:m].rearrange("p (r c) -> p r c", c=3)
        t_v = b[:, m : 2 * m].rearrange("p (r c) -> p r c", c=3)
        s = t % n_dbufs
        d_v = dring[:, s * m : (s + 1) * m].rearrange("p (c r) -> p r c", c=3)
        return p_v, t_v, d_v

    def dgroup(g, i):
        # component i of groups' d tiles: [128, group, r], unit inner stride
        s0 = (g * group) % n_dbufs
        w = dring[:, s0 * m : (s0 + group) * m]
        return w.rearrange("p (t c r) -> p t c r", t=group, c=3)[:, :, i, :]

    n_dve_sq = 3 - act_squares
    dve_ops_per_group = 3 + n_dve_sq
    # rotated dead-store scratch (2 groups deep per engine)
    dve_scrs = [
        nc.alloc_sbuf_tensor(f"dve_scr{k}", [P, group * r], bf16).ap()
        for k in range(2 * dve_ops_per_group)
    ]
    act_scrs = [
        nc.alloc_sbuf_tensor(f"act_scr{k}", [P, group * r], bf16).ap()
        for k in range(2 * act_squares)
    ]

    dma_sems = [nc.alloc_semaphore(f"dma_sem{i}") for i in range(n_bufs)]
    out_sem = nc.alloc_semaphore("out_sem")
    dve_sem = nc.alloc_semaphore("dve_sem")
    act_sem = nc.alloc_semaphore("act_sem") if act_squares else None

    # DVE emission order (as baseline): subs run ahead; group g's reduces
    # emitted after sub(4g+4) so their wait is stale when reached.
    dve_order = []
    for t in range(n_tiles):
        dve_order.append(("sub", t))
        if t % group == 0 and t >= group:
            dve_order.append(("red", t // group - 1))
    dve_order.append(("red", n_groups - 1))
    sub_done, redg_done = {}, {}
    v = 0
    for kind, x in dve_order:
        if kind == "sub":
            v += 1
            sub_done[x] = v
        else:
            v += dve_ops_per_group
            redg_done[x] = v

    # output flushes: all-but-last groups early, last group alone at the end
    chunks = [(0, n_groups - 1), (n_groups - 1, n_groups)]

    import contextlib

    @contextlib.contextmanager
    def _block():
        with nc.Block(no_gpsimd_drain=True) as blk:
            try:
                yield blk
            finally:
                if skip_exit_barrier:
                    nc.all_engine_barrier = lambda **kw: None
        if skip_exit_barrier:
            del nc.all_engine_barrier

    def emit_dma(eng, t):
        eng.dma_start(
            out=buf(t).rearrange("p (w m) -> p w m", w=2),
            in_=pt_v[t],
        ).then_inc(dma_sems[t % n_bufs], 16)

    with _block() as block:

        @block.sync
        def _(sync):
            for t in range(n_scalar_dmas, n_tiles):
                if t >= n_bufs:
                    # ring reuse: only the evicted tile's sub must be done
                    sync.wait_ge(dve_sem, sub_done[t - n_bufs])
                emit_dma(sync, t)
            n_out = 0
            for lo, hi in chunks:
                if act_squares:
                    sync.wait_ge(act_sem, act_squares * hi)
                sync.wait_ge(dve_sem, redg_done[hi - 1])
                sync.dma_start(
                    out=out[:, 6 * lo : 6 * hi], in_=acc[:, 6 * lo : 6 * hi]
                ).then_inc(out_sem, 16)
                n_out += 16
            sync.wait_ge(out_sem, n_out)

        @block.scalar
        def _(scalar):
            # early tile loads on the second HWDGE ring (ramp)
            for t in range(n_scalar_dmas):
                emit_dma(scalar, t)
            for g in range(n_groups):
                scalar.wait_ge(dve_sem, sub_done[(g + 1) * group - 1])
                if g >= 2:
                    scalar.wait_ge(act_sem, act_squares * (g - 1))
                for i in range(act_squares):
                    scalar.activation(
                        out=act_scrs[(g % 2) * act_squares + i][:].rearrange(
                            "p (t r) -> p t r", t=group
                        ),
                        in_=dgroup(g, i),
                        func=mybir.ActivationFunctionType.Square,
                        accum_out=acc[:, g * 6 + i : g * 6 + i + 1],
                    ).then_inc(act_sem, 1)

        @block.vector
        def _(vector):
            for kind, x in dve_order:
                if kind == "sub":
                    p_v, t_v, d_v = sub_views(x)
                    vector.wait_ge(dma_sems[x % n_bufs], 16 * (x // n_bufs + 1))
                    if x >= n_dbufs and act_squares:
                        # d-slot reuse: ACT squares of the evicted tile's
                        # group must be done (DVE's own reads are ordered
                        # by program order)
                        gp = (x - n_dbufs) // group
                        vector.wait_ge(act_sem, act_squares * (gp + 1))
                    vector.tensor_tensor(
                        out=d_v,
                        in0=p_v,
                        in1=t_v,
                        op=mybir.AluOpType.subtract,
                    ).then_inc(dve_sem, 1)
                else:
                    vector.wait_ge(dve_sem, sub_done[(x + 1) * group - 1])
                    ops = [(i, j, 3 + k) for k, (i, j) in enumerate(_PAIRS)]
                    ops += [(i, i, i) for i in range(act_squares, 3)]
                    for n, (i, j, col) in enumerate(ops):
                        vector.scalar_tensor_tensor(
                            out=dve_scrs[(x % 2) * dve_ops_per_group + n][:]
                            .rearrange("p (t r) -> p t r", t=group),
                            in0=dgroup(x, i),
                            scalar=1.0,
                            in1=dgroup(x, j),
                            op0=mybir.AluOpType.mult,
                            op1=mybir.AluOpType.mult,
                            accum_out=acc[:, x * 6 + col : x * 6 + col + 1],
                        ).then_inc(dve_sem, 1)

    nc.compile()
    return nc


_NC_CACHE: dict[tuple, object] = {}


def _get_nc(n_rows: int, n_tiles: int, use_act: bool, raw: bool = False,
            group: int = 4, **kw):
    key = (n_rows, n_tiles, use_act, raw, group, tuple(sorted(kw.items())))
    if key not in _NC_CACHE:
        if raw:
            _NC_CACHE[key] = build_gram_kernel_v2(n_rows, n_tiles, group=group, **kw)
        else:
            _NC_CACHE[key] = build_gram_kernel(n_rows, n_tiles, use_act)
    return _NC_CACHE[key]


def gram_from_partials(partials: np.ndarray, n_tiles: int | None = None) -> np.ndarray:
    """[..., 128, 6*slots] v2 partials -> full 3x3 Gram matrix (float64).

    v2 layout: group-major, col 6g+i = sq_i, col 6g+3+k = cross pair k.
    """
    slots = partials.shape[-1] // 6
    s = partials.astype(np.float64).reshape(-1, slots, 6).sum(axis=0).sum(axis=0)
    g = np.empty((3, 3), dtype=np.float64)
    g[0, 0], g[1, 1], g[2, 2] = s[0:3]
    for k, (i, j) in enumerate(_PAIRS):
        g[i, j] = g[j, i] = s[3 + k]
    return g


def run_device_partials(predictions: np.ndarray, targets: np.ndarray,
                        n_tiles: int = 4, use_act: bool = True,
                        raw: bool = False, group: int = 4, **run_kwargs):
    """Shard over N_CORES, run on device, return per-core partials + results."""
    b = predictions.shape[0]
    assert b % N_CORES == 0
    n_rows = b // N_CORES
    nc = _get_nc(n_rows, n_tiles, use_act, raw, group)
    preds = np.ascontiguousarray(predictions, dtype=np.float32).reshape(
        N_CORES, n_rows, 3
    )
    targs = np.ascontiguousarray(targets, dtype=np.float32).reshape(
        N_CORES, n_rows, 3
    )
    in_maps = [
        {"pt": np.stack([preds[c], targs[c]])} for c in range(N_CORES)
    ]
    res = run_bass_kernel_spmd(nc, in_maps, list(range(N_CORES)), **run_kwargs)
    partials = np.stack([r["partials"] for r in res.results])
    return partials, res


def kernel(predictions: np.ndarray, targets: np.ndarray, sigma: np.ndarray) -> np.ndarray:
    predictions = np.asarray(predictions, dtype=np.float32)
    targets = np.asarray(targets, dtype=np.float32)
    sigma64 = np.asarray(sigma, dtype=np.float64)

    partials, _ = run_device_partials(predictions, targets, n_tiles=32, raw=True)
    g = gram_from_partials(partials)

    sigma_inv = np.linalg.inv(sigma64)
    _, logdet = np.linalg.slogdet(sigma64)
    mean_mahal = float((sigma_inv * g).sum()) / predictions.shape[0]
    loss = abs(logdet + mean_mahal)
    return np.float32(loss)



# revision 6
# speedup vs baseline: 1.7808x; 1.7808x over previous
"""Trainium2 Bass kernel for CustomLossWithCovariance.

loss = abs(logdet(sigma) + mean_b[(p_b - t_b)^T sigma^{-1} (p_b - t_b)])

Only the 3x3 Gram matrix G = sum_b d_b d_b^T (d = pred - targ) requires
touching the [B, 3] data; the device computes per-core partial pair-sums
of G, and the host finishes with the tiny 3x3 algebra:
    mean_mahalanobis = <sigma_inv, G> / B
    loss = |logdet(sigma) + mean_mahalanobis|

Sharding: data-parallel over the batch across 8 NeuronCores (each core
streams a contiguous [B/8, 3] shard; partial sums gathered on host).

Per-core device kernel (raw Bacc, manual semaphores — see
build_gram_kernel_raw; build_gram_kernel is the Tile-framework
baseline kept for reference). Per tile of [128, 2M]:
  - DMA both halves (pred | targ) flat-contiguous in one dma_start
  - DVE: d = pred - targ, in-place into the pred half (unit-stride fp32)
  - ACT: Square(d_i) with accum_out -> per-partition sums of d_i^2
  - DVE: scalar_tensor_tensor(d_i * d_j, accum_out) -> cross sums
  (component APs are stride-3 views of the flat tiles, grouped 4 tiles
  per reduce instruction to amortize fixed costs)
"""

import numpy as np

import concourse.bass as bass
import concourse.bacc as bacc
import concourse.mybir as mybir
from concourse import tile
from concourse.bass_utils import run_bass_kernel_spmd

N_CORES = 8
B_FULL = 8388608
P = 128

_PAIRS = [(0, 1), (0, 2), (1, 2)]


def build_gram_kernel(n_rows: int, n_tiles: int, use_act: bool = True):
    """Build the per-core Bass module.

    Input: pt [2, n_rows, 3] f32 (pred stacked with targ)
    Output: partials [128, 6 * n_tiles] f32
        col t*3+i            : sum over this tile/partition of d_i^2
        col 3*n_tiles + t*3+k: sum of d_i*d_j for pair k in _PAIRS
    """
    assert n_rows % (P * n_tiles) == 0
    r = n_rows // (P * n_tiles)  # rows per partition per tile
    m = 3 * r                    # flat f32 elements per partition per tile
    f32 = mybir.dt.float32

    # Bacc (not plain Bass): its compile() pass legalizes semaphore waits
    # (each TRN2 instruction holds at most one wait slot).
    nc = bacc.Bacc("TRN2", target_bir_lowering=False, debug=False)
    pt = nc.dram_tensor("pt", [2, n_rows, 3], f32, kind="ExternalInput")
    out = nc.dram_tensor("partials", [P, 6 * n_tiles], f32, kind="ExternalOutput")

    # [t][p][w(2), m] — per tile/partition: pred chunk and targ chunk, each
    # m contiguous f32 in DRAM.
    pt_v = pt[:].rearrange("w (t p r) c -> t p w (r c)", t=n_tiles, p=P)

    with tile.TileContext(nc) as tc:
        with (
            tc.tile_pool(name="io", bufs=3) as io_pool,
            tc.tile_pool(name="dve_scr", bufs=2) as dve_scr,
            tc.tile_pool(name="act_scr", bufs=2) as act_scr,
            tc.tile_pool(name="acc", bufs=1) as acc_pool,
        ):
            acc_sq = acc_pool.tile([P, 3 * n_tiles], f32)
            acc_cr = acc_pool.tile([P, 3 * n_tiles], f32)

            for t in range(n_tiles):
                buf = io_pool.tile([P, 2 * m], f32, tag="buf")
                nc.sync.dma_start(
                    out=buf[:].rearrange("p (w m) -> p w m", w=2),
                    in_=pt_v[t],
                )

                # In-place: d = pred - targ, overwriting the pred half.
                nc.vector.tensor_tensor(
                    out=buf[:, 0:m],
                    in0=buf[:, 0:m],
                    in1=buf[:, m : 2 * m],
                    op=mybir.AluOpType.subtract,
                )
                d3 = buf[:, 0:m].rearrange("p (r c) -> p c r", c=3)

                # Diagonal sums on the scalar engine (Square + accum_out),
                # overlapping with the DVE cross-products.
                if use_act:
                    for i in range(3):
                        sq = act_scr.tile([P, r], f32, tag="sq")
                        nc.scalar.activation(
                            out=sq[:],
                            in_=d3[:, i, :],
                            func=mybir.ActivationFunctionType.Square,
                            accum_out=acc_sq[:, t * 3 + i : t * 3 + i + 1],
                        )
                else:
                    for i in range(3):
                        sq = dve_scr.tile([P, r], f32, tag="pr")
                        nc.vector.scalar_tensor_tensor(
                            out=sq[:],
                            in0=d3[:, i, :],
                            scalar=1.0,
                            in1=d3[:, i, :],
                            op0=mybir.AluOpType.mult,
                            op1=mybir.AluOpType.mult,
                            accum_out=acc_sq[:, t * 3 + i : t * 3 + i + 1],
                        )
                # Cross sums: fused multiply+reduce on DVE
                # (scalar_tensor_tensor: out = (in0 * 1.0) * in1, accum = sum).
                for k, (i, j) in enumerate(_PAIRS):
                    pr = dve_scr.tile([P, r], f32, tag="pr")
                    nc.vector.scalar_tensor_tensor(
                        out=pr[:],
                        in0=d3[:, i, :],
                        scalar=1.0,
                        in1=d3[:, j, :],
                        op0=mybir.AluOpType.mult,
                        op1=mybir.AluOpType.mult,
                        accum_out=acc_cr[:, t * 3 + k : t * 3 + k + 1],
                    )

            nc.sync.dma_start(out=out[:, 0 : 3 * n_tiles], in_=acc_sq[:])
            nc.sync.dma_start(out=out[:, 3 * n_tiles : 6 * n_tiles], in_=acc_cr[:])

    nc.compile()
    return nc


def build_gram_kernel_raw(n_rows: int, n_tiles: int = 32, n_bufs: int = 24,
                          group: int = 4, skip_exit_barrier: bool = True):
    """Raw-Bacc variant: manual semaphores, no TileContext.

    Skips Tile's prologue/epilogue (drain + two all-engine EVSEM
    barriers, ~16 us) — the only sync needed is a three-semaphore chain:
    DMA loads (one HWDGE ring) -> DVE -> ACT.

    The ring of tile buffers lives in ONE SBUF tensor so the fused
    multiply-reduces can span `group` consecutive tiles with a single
    instruction (free-dim AP [group, r]) — amortizing the per-op fixed
    cost and the accumulator-drain, which keeps both compute engines
    well under the DMA pace.

    Input: pt [2, n_rows, 3] f32. Output: partials [128, 6 * n_groups]
    (same slot layout as build_gram_kernel, with n_groups slots).
    """
    assert n_tiles % group == 0 and n_bufs % group == 0
    assert n_rows % (P * n_tiles) == 0
    n_groups = n_tiles // group
    r = n_rows // (P * n_tiles)
    m = 3 * r
    f32 = mybir.dt.float32

    nc = bacc.Bacc("TRN2", target_bir_lowering=False, debug=False)
    pt = nc.dram_tensor("pt", [2, n_rows, 3], f32, kind="ExternalInput")
    out = nc.dram_tensor("partials", [P, 6 * n_groups], f32, kind="ExternalOutput")
    pt_v = pt[:].rearrange("w (t p r) c -> t p w (r c)", t=n_tiles, p=P)

    ring = nc.alloc_sbuf_tensor("ring", [P, n_bufs * 2 * m], f32).ap()

    def buf(t):
        s = t % n_bufs
        return ring[:, s * 2 * m : (s + 1) * 2 * m]

    def dgroup(g, i):
        # component i of the diff halves of tiles 4g..4g+3: [128, group, r]
        s0 = (g * group) % n_bufs
        w = ring[:, s0 * 2 * m : (s0 + group) * 2 * m]
        return w.rearrange("p (t w r c) -> p t w c r", t=group, w=2, c=3)[:, :, 0, i, :]

    acc_sq = nc.alloc_sbuf_tensor("acc_sq", [P, 3 * n_groups], f32).ap()
    acc_cr = nc.alloc_sbuf_tensor("acc_cr", [P, 3 * n_groups], f32).ap()
    # Rotated scratch (dead stores of the fused ops), 2 groups deep so each
    # group's single stale semaphore wait also covers the scratch WAW from
    # two groups back.
    pr_scrs = [
        nc.alloc_sbuf_tensor(f"pr_scr{k}", [P, group * r], f32).ap() for k in range(6)
    ]
    sq_scrs = [
        nc.alloc_sbuf_tensor(f"sq_scr{k}", [P, group * r], f32).ap() for k in range(6)
    ]

    # One DMA-completion semaphore per ring buffer: a single shared sem
    # would be unsound — each dma_start is split across 16 SDMA engines
    # whose sub-completions interleave across in-flight DMAs.
    dma_sems = [nc.alloc_semaphore(f"dma_sem{i}") for i in range(n_bufs)]
    out_sem = nc.alloc_semaphore("out_sem")
    dve_sem = nc.alloc_semaphore("dve_sem")
    act_sem = nc.alloc_semaphore("act_sem")

    # DVE emission order: subs run ahead; the grouped multiply-reduces for
    # group g are emitted after sub(4g+4) so their drain-wait on the last
    # sub of the group is already satisfied when it executes (DVE writes
    # drain asynchronously). Only the last group trails the final sub.
    dve_order = []
    for t in range(n_tiles):
        dve_order.append(("sub", t))
        if t % group == 0 and t >= group:
            # one sub of stagger after the group's last sub
            dve_order.append(("stt", t // group - 1))
    dve_order.append(("stt", n_groups - 1))
    sub_done, sttg_done = {}, {}
    v = 0
    for kind, x in dve_order:
        if kind == "sub":
            v += 1
            sub_done[x] = v
        else:
            v += 3
            sttg_done[x] = v

    # Output chunks: flush finished accumulator columns while later tiles
    # still stream, so the tail only waits on the last small chunk.
    chunk = max(1, n_groups // 2)
    chunks = [(c, min(c + chunk, n_groups)) for c in range(0, n_groups, chunk)]

    import contextlib

    @contextlib.contextmanager
    def _block():
        # no_gpsimd_drain=True emits per-engine drains explicitly and then a
        # sem-only all-engine butterfly. The butterfly only delays NEFF end
        # (outputs are already fenced by the sequencer's out_sem wait), so
        # optionally no-op it during Block.__exit__.
        with nc.Block(no_gpsimd_drain=True) as blk:
            try:
                yield blk
            finally:
                if skip_exit_barrier:
                    nc.all_engine_barrier = lambda **kw: None
        if skip_exit_barrier:
            del nc.all_engine_barrier  # restore class method

    with _block() as block:

        @block.sync
        def _(sync):
            for t in range(n_tiles):
                if t >= n_bufs:
                    # ring reuse: all consumers of the buffer's previous
                    # occupant (tile t - n_bufs) must be done
                    prev = t - n_bufs
                    sync.wait_ge(dve_sem, sttg_done[prev // group])
                    sync.wait_ge(act_sem, 3 * (prev // group + 1))
                sync.dma_start(
                    out=buf(t).rearrange("p (w m) -> p w m", w=2),
                    in_=pt_v[t],
                ).then_inc(dma_sems[t % n_bufs], 16)
            n_out = 0
            for lo, hi in chunks:
                sync.wait_ge(act_sem, 3 * hi)
                sync.dma_start(
                    out=out[:, 3 * lo : 3 * hi], in_=acc_sq[:, 3 * lo : 3 * hi]
                ).then_inc(out_sem, 16)
                sync.wait_ge(dve_sem, sttg_done[hi - 1])
                sync.dma_start(
                    out=out[:, 3 * (n_groups + lo) : 3 * (n_groups + hi)],
                    in_=acc_cr[:, 3 * lo : 3 * hi],
                ).then_inc(out_sem, 16)
                n_out += 32
            sync.wait_ge(out_sem, n_out)

        @block.vector
        def _(vector):
            for kind, x in dve_order:
                if kind == "sub":
                    b = buf(x)
                    vector.wait_ge(dma_sems[x % n_bufs], 16 * (x // n_bufs + 1))
                    vector.tensor_tensor(
                        out=b[:, 0:m],
                        in0=b[:, 0:m],
                        in1=b[:, m : 2 * m],
                        op=mybir.AluOpType.subtract,
                    ).then_inc(dve_sem, 1)
                else:
                    vector.wait_ge(dve_sem, sub_done[(x + 1) * group - 1])
                    for k, (i, j) in enumerate(_PAIRS):
                        vector.scalar_tensor_tensor(
                            out=pr_scrs[(x % 2) * 3 + k][:].rearrange(
                                "p (t r) -> p t r", t=group
                            ),
                            in0=dgroup(x, i),
                            scalar=1.0,
                            in1=dgroup(x, j),
                            op0=mybir.AluOpType.mult,
                            op1=mybir.AluOpType.mult,
                            accum_out=acc_cr[:, x * 3 + k : x * 3 + k + 1],
                        ).then_inc(dve_sem, 1)

        @block.scalar
        def _(scalar):
            for g in range(n_groups):
                scalar.wait_ge(dve_sem, sub_done[(g + 1) * group - 1])
                if g >= 2:
                    # scratch slot reuse from two groups back
                    scalar.wait_ge(act_sem, 3 * (g - 1))
                for i in range(3):
                    scalar.activation(
                        out=sq_scrs[(g % 2) * 3 + i][:].rearrange(
                            "p (t r) -> p t r", t=group
                        ),
                        in_=dgroup(g, i),
                        func=mybir.ActivationFunctionType.Square,
                        accum_out=acc_sq[:, g * 3 + i : g * 3 + i + 1],
                    ).then_inc(act_sem, 1)

    nc.compile()
    return nc

def build_gram_kernel_v2(n_rows: int, n_tiles: int = 32, n_bufs: int = 24,
                         group: int = 4, n_dbufs: int = 12,
                         n_scalar_dmas: int = 8, act_squares: int = 3,
                         skip_exit_barrier: bool = True):
    """bf16-deinterleaved variant of build_gram_kernel_raw.

    The fp32 per-tile subtract writes d = pred - targ as bf16 with the
    three vector components DEINTERLEAVED (each component a unit-stride
    block) into a small d-ring.  The multiply-reduces then read bf16 at
    step 1, which unlocks the DVE 2x packed perf mode (fp32/stride-3 in
    the baseline capped DVE at ~85-103 elem/ns and made compute lag the
    402 GB/s DMA stream by ~8 us).

    Other deltas vs the baseline:
      - input-ring reuse only waits on the SUB of the evicted tile (the
        crosses read the d-ring, not the input ring), so the DMA queue
        never stalls on the reduce tail;
      - the first `n_scalar_dmas` tile loads are issued from the scalar
        engine's HWDGE ring in parallel with the sync engine's, halving
        the issue-rate-limited ramp;
      - the accumulator is laid out group-major ([sq0..2 cr0..2] per
        group) so each output flush is ONE dma, and the final flush
        covers only the last group's 6 columns.

    Output: partials [128, 6 * n_groups], col 6g+i = sum d_i^2 of group
    g for i<3, col 6g+3+k = sum d_i*d_j for pair k.
    """
    assert n_tiles % group == 0 and n_bufs % group == 0
    assert n_dbufs % group == 0 and n_dbufs >= 2 * group
    assert n_rows % (P * n_tiles) == 0
    assert 0 <= act_squares <= 3
    n_groups = n_tiles // group
    r = n_rows // (P * n_tiles)
    m = 3 * r
    f32, bf16 = mybir.dt.float32, mybir.dt.bfloat16

    nc = bacc.Bacc("TRN2", target_bir_lowering=False, debug=False)
    pt = nc.dram_tensor("pt", [2, n_rows, 3], f32, kind="ExternalInput")
    out = nc.dram_tensor("partials", [P, 6 * n_groups], f32, kind="ExternalOutput")
    pt_v = pt[:].rearrange("w (t p r) c -> t p w (r c)", t=n_tiles, p=P)

    ring = nc.alloc_sbuf_tensor("ring", [P, n_bufs * 2 * m], f32).ap()
    dring = nc.alloc_sbuf_tensor("dring", [P, n_dbufs * m], bf16).ap()
    acc = nc.alloc_sbuf_tensor("acc", [P, 6 * n_groups], f32).ap()

    def buf(t):
        s = t % n_bufs
        return ring[:, s * 2 * m : (s + 1) * 2 * m]

    def sub_views(t):
        b = buf(t)
        p_v = b[:, 0:m].rearrange("p (r c) -> p r c", c=3)
        t_v = b[:, m : 2 * m].rearrange("p (r c) -> p r c", c=3)
        s = t % n_dbufs
        d_v = dring[:, s * m : (s + 1) * m].rearrange("p (c r) -> p r c", c=3)
        return p_v, t_v, d_v

    def dgroup(g, i):
        # component i of groups' d tiles: [128, group, r], unit inner stride
        s0 = (g * group) % n_dbufs
        w = dring[:, s0 * m : (s0 + group) * m]
        return w.rearrange("p (t c r) -> p t c r", t=group, c=3)[:, :, i, :]

    n_dve_sq = 3 - act_squares
    dve_ops_per_group = 3 + n_dve_sq
    # rotated dead-store scratch (2 groups deep per engine)
    dve_scrs = [
        nc.alloc_sbuf_tensor(f"dve_scr{k}", [P, group * r], bf16).ap()
        for k in range(2 * dve_ops_per_group)
    ]
    act_scrs = [
        nc.alloc_sbuf_tensor(f"act_scr{k}", [P, group * r], bf16).ap()
        for k in range(2 * act_squares)
    ]

    dma_sems = [nc.alloc_semaphore(f"dma_sem{i}") for i in range(n_bufs)]
    out_sem = nc.alloc_semaphore("out_sem")
    dve_sem = nc.alloc_semaphore("dve_sem")
    act_sem = nc.alloc_semaphore("act_sem") if act_squares else None

    # DVE emission order (as baseline): subs run ahead; group g's reduces
    # emitted after sub(4g+4) so their wait is stale when reached.
    dve_order = []
    for t in range(n_tiles):
        dve_order.append(("sub", t))
        if t % group == 0 and t >= group:
            dve_order.append(("red", t // group - 1))
    dve_order.append(("red", n_groups - 1))
    sub_done, redg_done = {}, {}
    v = 0
    for kind, x in dve_order:
        if kind == "sub":
            v += 1
            sub_done[x] = v
        else:
            v += dve_ops_per_group
            redg_done[x] = v

    # output flushes: all-but-last groups early, last group alone at the end
    chunks = [(0, n_groups - 1), (n_groups - 1, n_groups)]

    import contextlib

    @contextlib.contextmanager
    def _block():
        with nc.Block(no_gpsimd_drain=True) as blk:
            try:
                yield blk
            finally:
                if skip_exit_barrier:
                    nc.all_engine_barrier = lambda **kw: None
        if skip_exit_barrier:
            del nc.all_engine_barrier

    def emit_dma(eng, t):
        eng.dma_start(
            out=buf(t).rearrange("p (w m) -> p w m", w=2),
            in_=pt_v[t],
        ).then_inc(dma_sems[t % n_bufs], 16)

    with _block() as block:

        @block.sync
        def _(sync):
            for t in range(n_scalar_dmas, n_tiles):
                if t >= n_bufs:
                    # ring reuse: only the evicted tile's sub must be done
                    sync.wait_ge(dve_sem, sub_done[t - n_bufs])
                emit_dma(sync, t)
            n_out = 0
            for lo, hi in chunks:
                if act_squares:
                    sync.wait_ge(act_sem, act_squares * hi)
                sync.wait_ge(dve_sem, redg_done[hi - 1])
                sync.dma_start(
                    out=out[:, 6 * lo : 6 * hi], in_=acc[:, 6 * lo : 6 * hi]
                ).then_inc(out_sem, 16)
                n_out += 16
            sync.wait_ge(out_sem, n_out)

        @block.scalar
        def _(scalar):
            # early tile loads on the second HWDGE ring (ramp)
            for t in range(n_scalar_dmas):
                emit_dma(scalar, t)
            for g in range(n_groups):
                scalar.wait_ge(dve_sem, sub_done[(g + 1) * group - 1])
                if g >= 2:
                    scalar.wait_ge(act_sem, act_squares * (g - 1))
                for i in range(act_squares):
                    scalar.activation(
                        out=act_scrs[(g % 2) * act_squares + i][:].rearrange(
                            "p (t r) -> p t r", t=group
                        ),
                        in_=dgroup(g, i),
                        func=mybir.ActivationFunctionType.Square,
                        accum_out=acc[:, g * 6 + i : g * 6 + i + 1],
                    ).then_inc(act_sem, 1)

        @block.vector
        def _(vector):
            for kind, x in dve_order:
                if kind == "sub":
                    p_v, t_v, d_v = sub_views(x)
                    vector.wait_ge(dma_sems[x % n_bufs], 16 * (x // n_bufs + 1))
                    if x >= n_dbufs and act_squares:
                        # d-slot reuse: ACT squares of the evicted tile's
                        # group must be done (DVE's own reads are ordered
                        # by program order)
                        gp = (x - n_dbufs) // group
                        vector.wait_ge(act_sem, act_squares * (gp + 1))
                    vector.tensor_tensor(
                        out=d_v,
                        in0=p_v,
                        in1=t_v,
                        op=mybir.AluOpType.subtract,
                    ).then_inc(dve_sem, 1)
                else:
                    vector.wait_ge(dve_sem, sub_done[(x + 1) * group - 1])
                    ops = [(i, j, 3 + k) for k, (i, j) in enumerate(_PAIRS)]
                    ops += [(i, i, i) for i in range(act_squares, 3)]
                    for n, (i, j, col) in enumerate(ops):
                        vector.scalar_tensor_tensor(
                            out=dve_scrs[(x % 2) * dve_ops_per_group + n][:]
                            .rearrange("p (t r) -> p t r", t=group),
                            in0=dgroup(x, i),
                            scalar=1.0,
                            in1=dgroup(x, j),
                            op0=mybir.AluOpType.mult,
                            op1=mybir.AluOpType.mult,
                            accum_out=acc[:, x * 6 + col : x * 6 + col + 1],
                        ).then_inc(dve_sem, 1)

    nc.compile()
    return nc


def build_gram_kernel_v3(n_rows: int, n_tiles: int = 32,
                         group_sizes: tuple = (4, 4, 4, 4, 4, 4, 4, 2, 1, 1),
                         skip_exit_barrier: bool = True):
    """Planar bf16 variant: host supplies component-planar tiles, the DMA
    casts fp32->bf16 in flight (SWDGE), and every on-chip operand is
    unit-stride bf16.

    Input pt [n_tiles, 128, 6r] f32, per (tile, partition) one contiguous
    chunk [p0 r | p1 r | p2 r | t0 r | t1 r | t2 r] (6 KB for r=256 — the
    descriptor sweet spot, ~413 GB/s vs 402 for the baseline's split
    chunks).  SWDGE (gpsimd-issued) DMA casts to bf16 on the fly — probe-
    measured at full read rate.  Per tile the DVE subtract then runs in
    2x packed mode (bf16, step 1): d = pred - targ IN-PLACE over the pred
    half.  Cross/square reduces read d unit-stride (no stride-3 penalty).

    Every tile has its own SBUF slot (32 x 3 KB bf16) and semaphore — no
    ring reuse, so the DMA stream never waits on compute.  Group sizes
    taper at the end so the after-last-DMA tail is only the final tile's
    sub + three N=256 reduces.

    Output: partials [128, 6 * n_groups]; col 6g+i = sum d_i^2, col
    6g+3+k = sum d_i d_j over group g's tiles.
    """
    assert sum(group_sizes) == n_tiles
    assert n_rows % (P * n_tiles) == 0
    n_groups = len(group_sizes)
    r = n_rows // (P * n_tiles)
    h = 3 * r  # bf16 elems per half-tile per partition
    f32, bf16 = mybir.dt.float32, mybir.dt.bfloat16
    max_g = max(group_sizes)

    ends = []
    e = -1
    for sz in group_sizes:
        e += sz
        ends.append(e)
    starts = [e - sz + 1 for e, sz in zip(ends, group_sizes)]

    nc = bacc.Bacc("TRN2", target_bir_lowering=False, debug=False)
    pt = nc.dram_tensor("pt", [n_tiles, P, 2 * h], f32, kind="ExternalInput")
    out = nc.dram_tensor("partials", [P, 6 * n_groups], f32, kind="ExternalOutput")
    pt_v = pt[:]

    ring = nc.alloc_sbuf_tensor("ring", [P, n_tiles * 2 * h], bf16).ap()
    acc = nc.alloc_sbuf_tensor("acc", [P, 6 * n_groups], f32).ap()

    def half(t, w):
        return ring[:, (2 * t + w) * h : (2 * t + w + 1) * h]

    def dgroup(g, i):
        # component i of group g's d (pred) halves: [128, size, r] step-1
        s = starts[g]
        w = ring[:, 2 * s * h : 2 * (s + group_sizes[g]) * h]
        return w.rearrange(
            "p (t w c r) -> p t w c r", t=group_sizes[g], w=2, c=3
        )[:, :, 0, i, :]

    dve_scrs = [
        nc.alloc_sbuf_tensor(f"dve_scr{k}", [P, max_g * r], bf16).ap()
        for k in range(6)
    ]
    act_scrs = [
        nc.alloc_sbuf_tensor(f"act_scr{k}", [P, max_g * r], bf16).ap()
        for k in range(6)
    ]

    dma_sems = [nc.alloc_semaphore(f"dma_sem{t}") for t in range(n_tiles)]
    out_sem = nc.alloc_semaphore("out_sem")
    dve_sem = nc.alloc_semaphore("dve_sem")
    act_sem = nc.alloc_semaphore("act_sem")

    # DVE order: subs run ahead, group reduces staggered one sub late.
    dve_order = []
    for t in range(n_tiles):
        dve_order.append(("sub", t))
        dve_order.extend(("red", g) for g in range(n_groups) if ends[g] == t - 1)
    dve_order.extend(("red", g) for g in range(n_groups) if ends[g] >= n_tiles - 1)
    sub_done, redg_done = {}, {}
    v = 0
    for kind, x in dve_order:
        if kind == "sub":
            v += 1
            sub_done[x] = v
        else:
            v += 3
            redg_done[x] = v

    chunks = [(0, n_groups - 1), (n_groups - 1, n_groups)]

    import contextlib

    @contextlib.contextmanager
    def _block():
        with nc.Block() as blk:
            try:
                yield blk
            finally:
                if skip_exit_barrier:
                    nc.all_engine_barrier = lambda **kw: None
        if skip_exit_barrier:
            del nc.all_engine_barrier

    with _block() as block:

        @block.gpsimd
        def _(gp):
            for t in range(n_tiles):
                gp.dma_start(
                    out=ring[:, 2 * t * h : 2 * (t + 1) * h],
                    in_=pt_v[t],
                ).then_inc(dma_sems[t], 16)

        @block.sync
        def _(sync):
            n_out = 0
            for lo, hi in chunks:
                sync.wait_ge(act_sem, 3 * hi)
                sync.wait_ge(dve_sem, redg_done[hi - 1])
                sync.dma_start(
                    out=out[:, 6 * lo : 6 * hi], in_=acc[:, 6 * lo : 6 * hi]
                ).then_inc(out_sem, 16)
                n_out += 16
            sync.wait_ge(out_sem, n_out)

        @block.scalar
        def _(scalar):
            for g in range(n_groups):
                scalar.wait_ge(dve_sem, sub_done[ends[g]])
                if g >= 2:
                    scalar.wait_ge(act_sem, 3 * (g - 1))
                for i in range(3):
                    scalar.activation(
                        out=act_scrs[(g % 2) * 3 + i][
                            :, : group_sizes[g] * r
                        ].rearrange("p (t r) -> p t r", t=group_sizes[g]),
                        in_=dgroup(g, i),
                        func=mybir.ActivationFunctionType.Square,
                        accum_out=acc[:, g * 6 + i : g * 6 + i + 1],
                    ).then_inc(act_sem, 1)

        @block.vector
        def _(vector):
            for kind, x in dve_order:
                if kind == "sub":
                    vector.wait_ge(dma_sems[x], 16)
                    vector.tensor_tensor(
                        out=half(x, 0),
                        in0=half(x, 0),
                        in1=half(x, 1),
                        op=mybir.AluOpType.subtract,
                    ).then_inc(dve_sem, 1)
                else:
                    vector.wait_ge(dve_sem, sub_done[ends[x]])
                    for k, (i, j) in enumerate(_PAIRS):
                        vector.scalar_tensor_tensor(
                            out=dve_scrs[(x % 2) * 3 + k][
                                :, : group_sizes[x] * r
                            ].rearrange("p (t r) -> p t r", t=group_sizes[x]),
                            in0=dgroup(x, i),
                            scalar=1.0,
                            in1=dgroup(x, j),
                            op0=mybir.AluOpType.mult,
                            op1=mybir.AluOpType.mult,
                            accum_out=acc[:, x * 6 + 3 + k : x * 6 + 4 + k],
                        ).then_inc(dve_sem, 1)

    nc.compile()
    return nc


def planarize(predictions: np.ndarray, targets: np.ndarray,
              n_tiles: int = 32) -> np.ndarray:
    """[B,3] pred/targ -> per-core planar tiles [cores, n_tiles, P, 6r] f32."""
    b = predictions.shape[0]
    n_rows = b // N_CORES
    r = n_rows // (P * n_tiles)
    out = np.empty((N_CORES, n_tiles, P, 6 * r), dtype=np.float32)
    pv = out[..., : 3 * r].reshape(N_CORES, n_tiles, P, 3, r)
    tv = out[..., 3 * r :].reshape(N_CORES, n_tiles, P, 3, r)
    pv[:] = np.asarray(predictions, dtype=np.float32).reshape(
        N_CORES, n_tiles, P, r, 3).transpose(0, 1, 2, 4, 3)
    tv[:] = np.asarray(targets, dtype=np.float32).reshape(
        N_CORES, n_tiles, P, r, 3).transpose(0, 1, 2, 4, 3)
    return out


_NC_CACHE: dict[tuple, object] = {}


def _get_nc(n_rows: int, n_tiles: int, use_act: bool, raw: bool = False,
            group: int = 4, **kw):
    key = (n_rows, n_tiles, use_act, raw, group, tuple(sorted(kw.items())))
    if key not in _NC_CACHE:
        if raw:
            _NC_CACHE[key] = build_gram_kernel_v3(n_rows, n_tiles, **kw)
        else:
            _NC_CACHE[key] = build_gram_kernel(n_rows, n_tiles, use_act)
    return _NC_CACHE[key]


def gram_from_partials(partials: np.ndarray, n_tiles: int | None = None) -> np.ndarray:
    """[..., 128, 6*slots] v2 partials -> full 3x3 Gram matrix (float64).

    v2 layout: group-major, col 6g+i = sq_i, col 6g+3+k = cross pair k.
    """
    slots = partials.shape[-1] // 6
    s = partials.astype(np.float64).reshape(-1, slots, 6).sum(axis=0).sum(axis=0)
    g = np.empty((3, 3), dtype=np.float64)
    g[0, 0], g[1, 1], g[2, 2] = s[0:3]
    for k, (i, j) in enumerate(_PAIRS):
        g[i, j] = g[j, i] = s[3 + k]
    return g


def run_device_partials(predictions: np.ndarray, targets: np.ndarray,
                        n_tiles: int = 32, use_act: bool = True,
                        raw: bool = True, group: int = 4, **run_kwargs):
    """Shard over N_CORES, run on device, return per-core partials + results."""
    b = predictions.shape[0]
    assert b % N_CORES == 0
    n_rows = b // N_CORES
    nc = _get_nc(n_rows, n_tiles, use_act, raw, group)
    planar = planarize(predictions, targets, n_tiles)
    in_maps = [{"pt": planar[c]} for c in range(N_CORES)]
    res = run_bass_kernel_spmd(nc, in_maps, list(range(N_CORES)), **run_kwargs)
    partials = np.stack([r["partials"] for r in res.results])
    return partials, res


def kernel(predictions: np.ndarray, targets: np.ndarray, sigma: np.ndarray) -> np.ndarray:
    predictions = np.asarray(predictions, dtype=np.float32)
    targets = np.asarray(targets, dtype=np.float32)
    sigma64 = np.asarray(sigma, dtype=np.float64)

    partials, _ = run_device_partials(predictions, targets, n_tiles=32, raw=True)
    g = gram_from_partials(partials)

    sigma_inv = np.linalg.inv(sigma64)
    _, logdet = np.linalg.slogdet(sigma64)
    mean_mahal = float((sigma_inv * g).sum()) / predictions.shape[0]
    loss = abs(logdet + mean_mahal)
    return np.float32(loss)

